# revision 26
# baseline (speedup 1.0000x reference)
"""Deformable Conv1D on 8 Trainium2 NeuronCores (Bass/Tile), batch data-parallel.

kernel(**inputs) takes the FULL inputs (x [16,4096,256] f32, w_off [5,256,5],
w_conv [5,256,512], b_conv [512]) and returns the FULL output [16,4096,512] f32.
Batch is sharded 2-per-core across 8 cores; no cross-core communication.

v2: the gather no longer uses transpose mode (which shredded each 512B row
into ~8 small hardware packets).  Rows are gathered contiguously into
[l, c] tiles and transposed to [c, l] on the PE (bf16 transposes), with the
PSUM->SBUF copies round-robined across the Scalar/Vector/Pool engines and the
main-conv matmuls software-pipelined one tile behind the transposes.

Per-core program:
  phase 1 (both batches up front): x -> SBUF; SWDGE cast-DMA writes bf16 x
    to DRAM (gather source); PE-transpose x -> xT fp32; offsets conv EXACTLY
    in fp32 (packed 25-wide stationary); DVE adds iota, clips, casts
    (truncating via the rtne -0.5 trick) to int16 indices; idx -> DRAM ->
    wrapped 16-partition x8-replica reload.
  phase 2 (per batch): for each 1024-l group, 5 no-transpose gathers land
    [128 l, 8, 256 c] bf16 tiles; per 128-l tile, 10 PE transposes make the
    [c, l] lhsT tiles, then 10-term PSUM accumulation against wconv bf16;
    DVE adds bias; DMA out.
"""

import sys

if '/opt/trn_rl_repo' not in sys.path:
    sys.path.insert(0, '/opt/trn_rl_repo')

from contextlib import ExitStack

import ml_dtypes
import numpy as np

import concourse.bass as bass
import concourse.tile as tile
from concourse import bacc, mybir
from concourse.bass_utils import run_bass_kernel_spmd

FP32 = mybir.dt.float32
BF16 = mybir.dt.bfloat16
I16 = mybir.dt.int16

B, L, C = 16, 4096, 256
F, K = 512, 5
NCORES = 8
BPC = B // NCORES  # batches per core


def build_kernel(tc, ins, outs, *, Bpc, L, C, F, K, cast_mode="rtne"):
    nc = tc.nc
    Cc = C // 128            # channel chunks (2)
    LT = L // 128            # l-tiles per batch (32)
    PAD = 4                  # zero margin around xT columns (taps reach +-2)
    WIN = 512                # offsets window width (one psum bank)
    OWN = WIN - (K - 1) - 2  # output cols owned per window (506)
    nwin = (L + OWN - 1) // OWN
    XTW = (nwin - 1) * OWN + WIN + PAD
    GRP = 1024               # gather group (l's per gather call)
    NG = L // GRP            # groups per batch (4)
    TPG = GRP // 128         # l-tiles per group (8)

    ctx = ExitStack()
    with ctx:
        const_p = ctx.enter_context(tc.tile_pool(name="const", bufs=1))
        x_p = ctx.enter_context(tc.tile_pool(name="x", bufs=1))
        xt_p = ctx.enter_context(tc.tile_pool(name="xt", bufs=1))
        xg_p = ctx.enter_context(tc.tile_pool(name="xg", bufs=2))
        xgt_p = ctx.enter_context(tc.tile_pool(name="xgt", bufs=4))
        idx_p = ctx.enter_context(tc.tile_pool(name="idx", bufs=2))
        posm_p = ctx.enter_context(tc.tile_pool(name="posm", bufs=1))
        acc_p = ctx.enter_context(tc.tile_pool(name="acc", bufs=1))
        out_p = ctx.enter_context(tc.tile_pool(name="out", bufs=3))
        ps_t = ctx.enter_context(tc.tile_pool(name="ps_t", bufs=2, space="PSUM"))
        ps_g = ctx.enter_context(tc.tile_pool(name="ps_g", bufs=2, space="PSUM"))
        ps_o = ctx.enter_context(tc.tile_pool(name="ps_o", bufs=2, space="PSUM"))
        ps_m = ctx.enter_context(tc.tile_pool(name="ps_m", bufs=2, space="PSUM"))
        dram_p = ctx.enter_context(tc.tile_pool(name="dram", bufs=1, space="DRAM"))

        # resident constants: w5a packs taps 0..3 at 32-aligned col groups
        # (w5a[c, 32t+k] = w_off[t, c, k]); w5b is tap 4.
        w5a_sb = const_p.tile([128, Cc, 101], FP32, tag="w5a")
        nc.sync.dma_start(w5a_sb[:], ins["w5a"][:].rearrange("j p k -> p j k"))
        w5b_sb = const_p.tile([128, Cc, K], FP32, tag="w5b")
        nc.sync.dma_start(w5b_sb[:], ins["w5b"][:].rearrange("j p k -> p j k"))
        wconv_sb = const_p.tile([128, K * Cc, F], BF16, tag="wconv")
        nc.sync.dma_start(wconv_sb[:], ins["wconv"][:].rearrange("q p f -> p q f"))
        bias_sb = const_p.tile([128, F], FP32, tag="bias")
        nc.sync.dma_start(bias_sb[:], ins["bias"][:])
        iota_sb = const_p.tile([K, L], FP32, tag="iota")
        nc.sync.dma_start(iota_sb[:], ins["iota"][:])
        ident_sb = const_p.tile([128, 128], FP32, tag="ident")
        nc.sync.dma_start(ident_sb[:], ins["ident"][:])
        identb_sb = const_p.tile([128, 128], BF16, tag="identb")
        nc.sync.dma_start(identb_sb[:], ins["identb"][:])
        sel16_sb = const_p.tile([128, 8, 128], FP32, tag="sel16")
        nc.sync.dma_start(sel16_sb[:], ins["sel16"][:].rearrange("g p m -> p g m"))

        xbf = dram_p.tile([Bpc, L, C], BF16, tag="xbf")

        idx_ts = {}

        # ---------------- phase 1: both batches ----------------
        for b in range(Bpc):
            # ---- load x[b] in halves: [128 (l%128), LT/2, C] ----
            xt = [xt_p.tile([128, XTW], FP32, tag=f"xt{j}", name=f"xt{j}_{b}")
                  for j in range(Cc)]
            for j in range(Cc):
                nc.vector.memset(xt[j][:, 0:PAD], 0.0)
                nc.vector.memset(xt[j][:, PAD + L:XTW], 0.0)
            LH = LT // 2
            for xh in range(2):
                x_sb = x_p.tile([128, LH, C], FP32, tag="x",
                                name=f"x_{b}_{xh}")
                nc.sync.dma_start(
                    x_sb[:],
                    ins["x"][b, xh * (L // 2):(xh + 1) * (L // 2)]
                    .rearrange("(t p) c -> p t c", p=128))
                # bf16 copy in DRAM (gather source), cast during DMA
                nc.gpsimd.dma_start(
                    out=xbf[b, xh * (L // 2):(xh + 1) * (L // 2)]
                    .rearrange("(t p) c -> p t c", p=128), in_=x_sb[:])
                # PE-transpose x -> xT[j][c, PAD + l] (fp32)
                for j in range(Cc):
                    for g4 in range(LH // 4):
                        g4g = xh * (LH // 4) + g4
                        pst4 = ps_t.tile([128, 4, 128], FP32, tag="pst")
                        for i4 in range(4):
                            nc.tensor.transpose(
                                pst4[:, i4, :],
                                x_sb[:, 4 * g4 + i4, j * 128:(j + 1) * 128],
                                ident_sb[:])
                        nc.scalar.copy(
                            xt[j][:, PAD + 512 * g4g:PAD + 512 * (g4g + 1)]
                            .rearrange("p (a c) -> p a c", a=4),
                            pst4[:])

            # ---- offsets windows -> idx [K, L] int16 (fp32-exact) ----
            # taps 0-3 land at psum partition groups {0,32,64,96}+[0,5);
            # tap 4 is accumulated onto tap 0's frame by a second matmul
            # whose moving slice is shifted +4 columns.
            posm_sb = posm_p.tile([K, L], FP32, tag="posm", name=f"posm_{b}")
            for s in range(nwin):
                o = s * OWN
                W = min(OWN, L - o)
                ps = ps_o.tile([128, WIN], FP32, tag="ps25")
                for j in range(Cc):
                    nc.tensor.matmul(
                        ps[0:101, :], w5a_sb[:, j, :], xt[j][:, o:o + WIN],
                        start=(j == 0), stop=(j == Cc - 1))
                for j in range(Cc):
                    nc.tensor.matmul(
                        ps[0:K, 0:WIN - 4], w5b_sb[:, j, :],
                        xt[j][:, o + 4:o + WIN], start=False,
                        stop=(j == Cc - 1), skip_group_check=True)
                acc = acc_p.tile([K, OWN], FP32, tag="acc")
                nc.vector.tensor_tensor(
                    out=acc[:, :W], in0=iota_sb[:, o:o + W],
                    in1=ps[0:K, 2:2 + W], op=mybir.AluOpType.add)
                for t, sh in ((32, 3), (64, 4), (96, 5)):
                    nc.vector.tensor_tensor(
                        out=acc[:, :W], in0=acc[:, :W],
                        in1=ps[t:t + K, sh:sh + W], op=mybir.AluOpType.add)
                if cast_mode == "rtne":
                    # HW float->int cast rounds to nearest even; emulate the
                    # reference's truncation via floor = rtne(clip(pos) - 0.5).
                    # Asymmetric clip bounds keep the clipped values off the
                    # rounding-half boundaries (0.25-0.5=-0.25 -> 0;
                    # (L-1)+0.25-0.5 -> L-1).  Kept in fp32 here; the int16
                    # cast happens after the wrapped reload.
                    nc.vector.tensor_scalar(
                        out=acc[:, :W], in0=acc[:, :W],
                        scalar1=0.25, scalar2=float(L - 1) + 0.25,
                        op0=mybir.AluOpType.max, op1=mybir.AluOpType.min)
                    nc.vector.tensor_scalar(
                        out=posm_sb[:, o:o + W], in0=acc[:, :W],
                        scalar1=-0.5, scalar2=None, op0=mybir.AluOpType.add)
                else:
                    # CoreSim float->int cast truncates toward zero.
                    nc.vector.tensor_scalar(
                        out=posm_sb[:, o:o + W], in0=acc[:, :W],
                        scalar1=0.0, scalar2=float(L - 1),
                        op0=mybir.AluOpType.max, op1=mybir.AluOpType.min)

            # ---- positions -> gather-index tiles, all on-chip ----
            # 1) PE-transpose posm [5, L] in 128-chunks -> U [128 (l%128), K, LT]
            u_sb = idx_p.tile([128, K, LT], FP32, tag="u", name=f"u_{b}")
            for t4 in range(LT // 4):
                psu = ps_t.tile([128, 4, 128], FP32, tag="pst",
                                name=f"psu_{b}_{t4}")
                for i4 in range(4):
                    nc.tensor.transpose(
                        psu[:, i4, 0:K],
                        posm_sb[:, (4 * t4 + i4) * 128:(4 * t4 + i4 + 1) * 128],
                        ident_sb[0:K, 0:K])
                nc.scalar.copy(
                    u_sb[:, :, 4 * t4:4 * (t4 + 1)],
                    psu[:, 0:4, 0:K].rearrange("p a k -> p k a"))
            # 2) per (g8 residue group, h half): replicate U's 16-partition
            #    slice to 128 partitions via sel16 matmul (exact 1.0*v), then
            #    cast to int16 (rtne -0.5 trick already applied in posm).
            for g8 in range(8):
                for q in range(4):
                    psr = ps_t.tile([128, 4, 128], FP32, tag="pst",
                                    name=f"psr_{b}_{g8}_{q}")
                    nc.tensor.matmul(
                        psr[:, 0, 0:K * 8],
                        sel16_sb[:, g8, :],
                        u_sb[:, :, 8 * q:8 * (q + 1)],
                        start=True, stop=True)
                    idx_j = idx_p.tile([128, K * 8], I16, tag=f"idxJ{g8}",
                                       name=f"idxJ{g8}_{q}_{b}")
                    nc.vector.tensor_copy(idx_j[:], psr[:, 0, 0:K * 8])
                    idx_ts[b, g8, q] = idx_j

        # ---------------- phase 2: gather + transpose + main conv ----------------
        # Gather (g8, h): num_idxs = 1280 covers taps k=0..4 x t' in
        # [16h, 16h+16); descriptor i = 256k + 16t' + pp targets row
        # l = 128(16h + t') + 16 g8 + pp, landing at out[16(t'%8) + pp,
        # slot 2k + t'//8, :].  Tile (g8, j): rows l = 1024j + 128uu +
        # 16 g8 + pp at partition 16uu + pp, from slot 2k + j%2 of half j//2.
        NIDX = K * 16 * 8           # 640 indices per gather call
        NSLOT = NIDX // 128         # 5 output slots (= taps)

        def emit_gathers(b, h):
            xg = []
            for g8 in range(8):
                xgk = xg_p.tile([128, NSLOT, C], BF16, tag=f"xg{g8}",
                                name=f"xg{g8}_{b}_{h}")
                nc.gpsimd.dma_gather(
                    out_ap=xgk[:], in_ap=xbf[b],
                    idxs_ap=idx_ts[b, g8, h][:],
                    num_idxs=NIDX, num_idxs_reg=NIDX,
                    elem_size=C, transpose=False, single_packet=True,
                    queue_num=0)
                xg.append(xgk)
            return xg

        def emit_transposes(b, q, g8):
            xg = cur_xg[q % 2][g8]
            ts = []
            for half in range(2):
                pg = ps_g.tile([128, K, 128], BF16, tag="psg")
                for m5 in range(K):
                    m = half * K + m5
                    k, j2 = m // Cc, m % Cc
                    nc.tensor.transpose(
                        pg[:, m5, :],
                        xg[:, k, j2 * 128:(j2 + 1) * 128],
                        identb_sb[:])
                xgt = xgt_p.tile([128, K, 128], BF16, tag="xgt",
                                 name=f"xgt{half}_{b}_{g8}_{q}")
                if half == 0:
                    nc.scalar.copy(xgt[:], pg[:])
                else:
                    nc.vector.tensor_copy(xgt[:], pg[:])
                ts.append(xgt)
            return ts

        out_v = outs["out"]

        def emit_matmuls(b, g8, j, ts):
            pso = ps_m.tile([128, F], FP32, tag="pso")
            for m in range(K * Cc):
                nc.tensor.matmul(
                    pso[:], ts[m // K][:, m % K, :], wconv_sb[:, m, :],
                    start=(m == 0), stop=(m == K * Cc - 1))
            o_sb = out_p.tile([128, F], FP32, tag="osb")
            nc.vector.tensor_tensor(
                out=o_sb[:], in0=pso[:], in1=bias_sb[:],
                op=mybir.AluOpType.add)
            dst = out_v[b].rearrange(
                "(j uu g pp) f -> j g uu pp f", uu=8, g=8, pp=16)[j, g8]
            nc.sync.dma_start(dst, o_sb[:])

        for b in range(Bpc):
            cur_xg = [None, None]
            cur_xg[0] = emit_gathers(b, 0)
            pending = None  # (g8, j, transposed tiles)
            NQ = LT // 8    # quarters per batch (4)
            for q in range(NQ):
                if q + 1 < NQ:
                    cur_xg[(q + 1) % 2] = emit_gathers(b, q + 1)
                for g8 in range(8):
                    ts = emit_transposes(b, q, g8)
                    if pending is not None:
                        emit_matmuls(b, pending[0], pending[1], pending[2])
                    pending = (g8, q, ts)
            emit_matmuls(b, pending[0], pending[1], pending[2])


_CACHE = {}


def _build_program(cast_mode="rtne"):
    nc = bacc.Bacc("TRN2", target_bir_lowering=False, debug=False,
                   num_devices=NCORES, num_swdge_queues=4)
    Cc = C // 128
    ins = {
        "x": nc.dram_tensor("x", [BPC, L, C], FP32, kind="ExternalInput").ap(),
        "w5a": nc.dram_tensor("w5a", [Cc, 128, 101], FP32,
                              kind="ExternalInput").ap(),
        "w5b": nc.dram_tensor("w5b", [Cc, 128, K], FP32,
                              kind="ExternalInput").ap(),
        "wconv": nc.dram_tensor("wconv", [K * Cc, 128, F], BF16,
                                kind="ExternalInput").ap(),
        "bias": nc.dram_tensor("bias", [128, F], FP32,
                               kind="ExternalInput").ap(),
        "ident": nc.dram_tensor("ident", [128, 128], FP32,
                                kind="ExternalInput").ap(),
        "identb": nc.dram_tensor("identb", [128, 128], BF16,
                                 kind="ExternalInput").ap(),
        "iota": nc.dram_tensor("iota", [K, L], FP32,
                               kind="ExternalInput").ap(),
        "sel16": nc.dram_tensor("sel16", [8, 128, 128], FP32,
                                kind="ExternalInput").ap(),
    }
    outs = {
        "out": nc.dram_tensor("out", [BPC, L, F], FP32,
                              kind="ExternalOutput").ap(),
    }
    with tile.TileContext(nc) as tc:
        build_kernel(tc, ins, outs, Bpc=BPC, L=L, C=C, F=F, K=K,
                     cast_mode=cast_mode)
    nc.compile()
    return nc


def _prep_consts(w_off, w_conv, b_conv):
    Cc = C // 128
    w5a = np.zeros((Cc, 128, 101), np.float32)
    for t in range(4):
        for j in range(Cc):
            w5a[j, :, 32 * t:32 * t + K] = w_off[t, j * 128:(j + 1) * 128, :]
    w5b = np.zeros((Cc, 128, K), np.float32)
    for j in range(Cc):
        w5b[j] = w_off[4, j * 128:(j + 1) * 128, :]
    wconv = np.zeros((K * Cc, 128, F), ml_dtypes.bfloat16)
    for k in range(K):
        for j in range(Cc):
            wconv[k * Cc + j] = w_conv[k, j * 128:(j + 1) * 128, :].astype(
                ml_dtypes.bfloat16)
    return {
        "w5a": w5a,
        "w5b": w5b,
        "wconv": wconv,
        "bias": np.broadcast_to(
            np.asarray(b_conv, np.float32)[None, :], (128, F)).copy(),
        "iota": np.broadcast_to(
            np.arange(L, dtype=np.float32)[None, :], (K, L)).copy(),
        "ident": np.eye(128, dtype=np.float32),
        "identb": np.eye(128, dtype=ml_dtypes.bfloat16),
        "sel16": (np.arange(128)[None, :, None]
                  == 16 * np.arange(8)[:, None, None]
                  + np.arange(128)[None, None, :] % 16).astype(np.float32),
    }


def run(x, w_off, w_conv, b_conv, trace=False, trace_kwargs=None):
    x = np.ascontiguousarray(np.asarray(x, np.float32))
    assert x.shape == (B, L, C), x.shape
    if "nc" not in _CACHE:
        _CACHE["nc"] = _build_program()
    nc = _CACHE["nc"]
    consts = _prep_consts(np.asarray(w_off, np.float32),
                          np.asarray(w_conv, np.float32),
                          np.asarray(b_conv, np.float32))
    in_maps = [
        {"x": np.ascontiguousarray(x[i * BPC:(i + 1) * BPC]), **consts}
        for i in range(NCORES)
    ]
    res = run_bass_kernel_spmd(nc, in_maps, list(range(NCORES)),
                               trace=trace, **(trace_kwargs or {}))
    _CACHE["last"] = res
    out = np.concatenate([res.results[i]["out"] for i in range(NCORES)], axis=0)
    return np.ascontiguousarray(out.astype(np.float32))


def kernel(x, w_off, w_conv, b_conv):
    return run(x, w_off, w_conv, b_conv)


# revision 27
# speedup vs baseline: 1.3584x; 1.3584x over previous
"""Deformable Conv1D on 8 Trainium2 NeuronCores (Bass/Tile), batch data-parallel.

kernel(**inputs) takes the FULL inputs (x [16,4096,256] f32, w_off [5,256,5],
w_conv [5,256,512], b_conv [512]) and returns the FULL output [16,4096,512] f32.
Batch is sharded 2-per-core across 8 cores; no cross-core communication.

v2: the gather no longer uses transpose mode (which shredded each 512B row
into ~8 small hardware packets).  Rows are gathered contiguously into
[l, c] tiles and transposed to [c, l] on the PE (bf16 transposes), with the
PSUM->SBUF copies round-robined across the Scalar/Vector/Pool engines and the
main-conv matmuls software-pipelined one tile behind the transposes.

Per-core program:
  phase 1 (both batches up front): x -> SBUF; SWDGE cast-DMA writes bf16 x
    to DRAM (gather source); PE-transpose x -> xT fp32; offsets conv EXACTLY
    in fp32 (packed 25-wide stationary); DVE adds iota, clips, casts
    (truncating via the rtne -0.5 trick) to int16 indices; idx -> DRAM ->
    wrapped 16-partition x8-replica reload.
  phase 2 (per batch): for each 1024-l group, 5 no-transpose gathers land
    [128 l, 8, 256 c] bf16 tiles; per 128-l tile, 10 PE transposes make the
    [c, l] lhsT tiles, then 10-term PSUM accumulation against wconv bf16;
    DVE adds bias; DMA out.
"""

import sys

if '/opt/trn_rl_repo' not in sys.path:
    sys.path.insert(0, '/opt/trn_rl_repo')

from contextlib import ExitStack

import ml_dtypes
import numpy as np

import concourse.bass as bass
import concourse.tile as tile
from concourse import bacc, mybir
from concourse.bass_utils import run_bass_kernel_spmd

FP32 = mybir.dt.float32
BF16 = mybir.dt.bfloat16
I16 = mybir.dt.int16

B, L, C = 16, 4096, 256
F, K = 512, 5
NCORES = 8
BPC = B // NCORES  # batches per core


def build_kernel(tc, ins, outs, *, Bpc, L, C, F, K, cast_mode="rtne"):
    nc = tc.nc
    Cc = C // 128            # channel chunks (2)
    LT = L // 128            # l-tiles per batch (32)
    PAD = 4                  # zero margin around xT columns (taps reach +-2)
    WIN = 512                # offsets window width (one psum bank)
    OWN = WIN - (K - 1) - 2  # output cols owned per window (506)
    nwin = (L + OWN - 1) // OWN
    XTW = (nwin - 1) * OWN + WIN + PAD
    GRP = 1024               # gather group (l's per gather call)
    NG = L // GRP            # groups per batch (4)
    TPG = GRP // 128         # l-tiles per group (8)

    ctx = ExitStack()
    with ctx:
        const_p = ctx.enter_context(tc.tile_pool(name="const", bufs=1))
        x_p = ctx.enter_context(tc.tile_pool(name="x", bufs=1))
        xt_p = ctx.enter_context(tc.tile_pool(name="xt", bufs=1))
        xg_p = ctx.enter_context(tc.tile_pool(name="xg", bufs=2))
        xgt_p = ctx.enter_context(tc.tile_pool(name="xgt", bufs=6))
        idx_p = ctx.enter_context(tc.tile_pool(name="idx", bufs=2))
        posm_p = ctx.enter_context(tc.tile_pool(name="posm", bufs=1))
        acc_p = ctx.enter_context(tc.tile_pool(name="acc", bufs=2))
        out_p = ctx.enter_context(tc.tile_pool(name="out", bufs=3))
        ps_t = ctx.enter_context(tc.tile_pool(name="ps_t", bufs=2, space="PSUM"))
        ps_g = ctx.enter_context(tc.tile_pool(name="ps_g", bufs=2, space="PSUM"))
        ps_o = ctx.enter_context(tc.tile_pool(name="ps_o", bufs=2, space="PSUM"))
        ps_m = ctx.enter_context(tc.tile_pool(name="ps_m", bufs=2, space="PSUM"))
        dram_p = ctx.enter_context(tc.tile_pool(name="dram", bufs=1, space="DRAM"))

        # resident constants: w5a packs taps 0..3 at 32-aligned col groups
        # (w5a[c, 32t+k] = w_off[t, c, k]); w5b is tap 4.
        w5a_sb = const_p.tile([128, Cc, 101], FP32, tag="w5a")
        nc.sync.dma_start(w5a_sb[:], ins["w5a"][:].rearrange("j p k -> p j k"))
        w5b_sb = const_p.tile([128, Cc, K], FP32, tag="w5b")
        nc.sync.dma_start(w5b_sb[:], ins["w5b"][:].rearrange("j p k -> p j k"))
        wconv_sb = const_p.tile([128, K * Cc, F], BF16, tag="wconv")
        nc.sync.dma_start(wconv_sb[:], ins["wconv"][:].rearrange("q p f -> p q f"))
        bias_sb = const_p.tile([128, F], FP32, tag="bias")
        nc.sync.dma_start(bias_sb[:], ins["bias"][:])
        iota_sb = const_p.tile([K, L], FP32, tag="iota")
        nc.sync.dma_start(iota_sb[:], ins["iota"][:])
        ident_sb = const_p.tile([128, 128], FP32, tag="ident")
        nc.sync.dma_start(ident_sb[:], ins["ident"][:])
        identb_sb = const_p.tile([128, 128], BF16, tag="identb")
        nc.sync.dma_start(identb_sb[:], ins["identb"][:])
        sel16_sb = const_p.tile([128, 8, 128], FP32, tag="sel16")
        nc.sync.dma_start(sel16_sb[:], ins["sel16"][:].rearrange("g p m -> p g m"))

        xbf = dram_p.tile([Bpc, L, C], BF16, tag="xbf")

        idx_ts = {}

        # ---------------- phase 1: both batches ----------------
        for b in range(Bpc):
            # ---- load x[b] in halves: [128 (l%128), LT/2, C] ----
            xt = [xt_p.tile([128, XTW], FP32, tag=f"xt{j}", name=f"xt{j}_{b}")
                  for j in range(Cc)]
            for j in range(Cc):
                nc.vector.memset(xt[j][:, 0:PAD], 0.0)
                nc.vector.memset(xt[j][:, PAD + L:XTW], 0.0)
            LH = LT // 2
            for xh in range(2):
                x_sb = x_p.tile([128, LH, C], FP32, tag="x",
                                name=f"x_{b}_{xh}")
                nc.sync.dma_start(
                    x_sb[:],
                    ins["x"][b, xh * (L // 2):(xh + 1) * (L // 2)]
                    .rearrange("(t p) c -> p t c", p=128))
                # bf16 copy in DRAM (gather source), cast during DMA
                nc.gpsimd.dma_start(
                    out=xbf[b, xh * (L // 2):(xh + 1) * (L // 2)]
                    .rearrange("(t p) c -> p t c", p=128), in_=x_sb[:])
                # PE-transpose x -> xT[j][c, PAD + l] (fp32)
                for j in range(Cc):
                    for g4 in range(LH // 4):
                        g4g = xh * (LH // 4) + g4
                        pst4 = ps_t.tile([128, 4, 128], FP32, tag="pst")
                        for i4 in range(4):
                            nc.tensor.transpose(
                                pst4[:, i4, :],
                                x_sb[:, 4 * g4 + i4, j * 128:(j + 1) * 128],
                                ident_sb[:])
                        nc.scalar.copy(
                            xt[j][:, PAD + 512 * g4g:PAD + 512 * (g4g + 1)]
                            .rearrange("p (a c) -> p a c", a=4),
                            pst4[:])

            # ---- offsets windows -> idx [K, L] int16 (fp32-exact) ----
            # taps 0-3 land at psum partition groups {0,32,64,96}+[0,5);
            # tap 4 is accumulated onto tap 0's frame by a second matmul
            # whose moving slice is shifted +4 columns.
            posm_sb = posm_p.tile([K, L], FP32, tag="posm", name=f"posm_{b}")
            for s in range(nwin):
                o = s * OWN
                W = min(OWN, L - o)
                ps = ps_o.tile([128, WIN], FP32, tag="ps25")
                for j in range(Cc):
                    nc.tensor.matmul(
                        ps[0:101, :], w5a_sb[:, j, :], xt[j][:, o:o + WIN],
                        start=(j == 0), stop=(j == Cc - 1))
                for j in range(Cc):
                    nc.tensor.matmul(
                        ps[0:K, 0:WIN - 4], w5b_sb[:, j, :],
                        xt[j][:, o + 4:o + WIN], start=False,
                        stop=(j == Cc - 1), skip_group_check=True)
                acc = acc_p.tile([K, OWN], FP32, tag="acc")
                nc.vector.tensor_tensor(
                    out=acc[:, :W], in0=iota_sb[:, o:o + W],
                    in1=ps[0:K, 2:2 + W], op=mybir.AluOpType.add)
                for t, sh in ((32, 3), (64, 4), (96, 5)):
                    nc.vector.tensor_tensor(
                        out=acc[:, :W], in0=acc[:, :W],
                        in1=ps[t:t + K, sh:sh + W], op=mybir.AluOpType.add)
                if cast_mode == "rtne":
                    # HW float->int cast rounds to nearest even; emulate the
                    # reference's truncation via floor = rtne(clip(pos) - 0.5).
                    # Asymmetric clip bounds keep the clipped values off the
                    # rounding-half boundaries (0.25-0.5=-0.25 -> 0;
                    # (L-1)+0.25-0.5 -> L-1).  Kept in fp32 here; the int16
                    # cast happens after the wrapped reload.
                    nc.vector.tensor_scalar(
                        out=acc[:, :W], in0=acc[:, :W],
                        scalar1=0.25, scalar2=float(L - 1) + 0.25,
                        op0=mybir.AluOpType.max, op1=mybir.AluOpType.min)
                    nc.vector.tensor_scalar(
                        out=posm_sb[:, o:o + W], in0=acc[:, :W],
                        scalar1=-0.5, scalar2=None, op0=mybir.AluOpType.add)
                else:
                    # CoreSim float->int cast truncates toward zero.
                    nc.vector.tensor_scalar(
                        out=posm_sb[:, o:o + W], in0=acc[:, :W],
                        scalar1=0.0, scalar2=float(L - 1),
                        op0=mybir.AluOpType.max, op1=mybir.AluOpType.min)

            # ---- positions -> gather-index tiles, all on-chip ----
            # 1) PE-transpose posm [5, L] in 128-chunks -> U [128 (l%128), K, LT]
            u_sb = idx_p.tile([128, K, LT], FP32, tag="u", name=f"u_{b}")
            for t4 in range(LT // 4):
                psu = ps_t.tile([128, 4, 128], FP32, tag="pst",
                                name=f"psu_{b}_{t4}")
                for i4 in range(4):
                    nc.tensor.transpose(
                        psu[:, i4, 0:K],
                        posm_sb[:, (4 * t4 + i4) * 128:(4 * t4 + i4 + 1) * 128],
                        ident_sb[0:K, 0:K])
                nc.scalar.copy(
                    u_sb[:, :, 4 * t4:4 * (t4 + 1)],
                    psu[:, 0:4, 0:K].rearrange("p a k -> p k a"))
            # 2) per (g8 residue group, h half): replicate U's 16-partition
            #    slice to 128 partitions via sel16 matmul (exact 1.0*v), then
            #    cast to int16 (rtne -0.5 trick already applied in posm).
            for g8 in range(8):
                for q in range(4):
                    psr = ps_t.tile([128, 4, 128], FP32, tag="pst",
                                    name=f"psr_{b}_{g8}_{q}")
                    nc.tensor.matmul(
                        psr[:, 0, 0:K * 8],
                        sel16_sb[:, g8, :],
                        u_sb[:, :, 8 * q:8 * (q + 1)],
                        start=True, stop=True)
                    idx_j = idx_p.tile([128, K * 8], I16, tag=f"idxJ{g8}_{q}",
                                       name=f"idxJ{g8}_{q}_{b}")
                    nc.vector.tensor_copy(idx_j[:], psr[:, 0, 0:K * 8])
                    idx_ts[b, g8, q] = idx_j

        # ---------------- phase 2: gather + transpose + main conv ----------------
        # Gather (g8, h): num_idxs = 1280 covers taps k=0..4 x t' in
        # [16h, 16h+16); descriptor i = 256k + 16t' + pp targets row
        # l = 128(16h + t') + 16 g8 + pp, landing at out[16(t'%8) + pp,
        # slot 2k + t'//8, :].  Tile (g8, j): rows l = 1024j + 128uu +
        # 16 g8 + pp at partition 16uu + pp, from slot 2k + j%2 of half j//2.
        NIDX = K * 16 * 8           # 640 indices per gather call
        NSLOT = NIDX // 128         # 5 output slots (= taps)

        def emit_gathers(b, h):
            xg = []
            for g8 in range(8):
                xgk = xg_p.tile([128, NSLOT, C], BF16, tag=f"xg{g8}",
                                name=f"xg{g8}_{b}_{h}")
                nc.gpsimd.dma_gather(
                    out_ap=xgk[:], in_ap=xbf[b],
                    idxs_ap=idx_ts[b, g8, h][:],
                    num_idxs=NIDX, num_idxs_reg=NIDX,
                    elem_size=C, transpose=False, single_packet=True,
                    queue_num=0)
                xg.append(xgk)
            return xg

        def emit_transposes(b, q, g8):
            xg = cur_xg[q % 2][g8]
            ts = []
            for half in range(2):
                pg = ps_g.tile([128, K, 128], BF16, tag="psg")
                for m5 in range(K):
                    m = half * K + m5
                    k, j2 = m // Cc, m % Cc
                    nc.tensor.transpose(
                        pg[:, m5, :],
                        xg[:, k, j2 * 128:(j2 + 1) * 128],
                        identb_sb[:])
                xgt = xgt_p.tile([128, K, 128], BF16, tag="xgt",
                                 name=f"xgt{half}_{b}_{g8}_{q}")
                if half == 0:
                    nc.scalar.copy(xgt[:], pg[:])
                else:
                    nc.vector.tensor_copy(xgt[:], pg[:])
                ts.append(xgt)
            return ts

        out_v = outs["out"]

        def emit_matmuls(b, g8, j, ts):
            pso = ps_m.tile([128, F], FP32, tag="pso")
            for m in range(K * Cc):
                nc.tensor.matmul(
                    pso[:], ts[m // K][:, m % K, :], wconv_sb[:, m, :],
                    start=(m == 0), stop=(m == K * Cc - 1))
            o_sb = out_p.tile([128, F], FP32, tag="osb")
            nc.vector.tensor_tensor(
                out=o_sb[:], in0=pso[:], in1=bias_sb[:],
                op=mybir.AluOpType.add)
            dst = out_v[b].rearrange(
                "(j uu g pp) f -> j g uu pp f", uu=8, g=8, pp=16)[j, g8]
            nc.sync.dma_start(dst, o_sb[:])

        for b in range(Bpc):
            cur_xg = [None, None]
            cur_xg[0] = emit_gathers(b, 0)
            pending = None  # (g8, j, transposed tiles)
            NQ = LT // 8    # quarters per batch (4)
            for q in range(NQ):
                if q + 1 < NQ:
                    cur_xg[(q + 1) % 2] = emit_gathers(b, q + 1)
                for g8 in range(8):
                    ts = emit_transposes(b, q, g8)
                    if pending is not None:
                        emit_matmuls(b, pending[0], pending[1], pending[2])
                    pending = (g8, q, ts)
            emit_matmuls(b, pending[0], pending[1], pending[2])


_CACHE = {}


def _build_program(cast_mode="rtne"):
    nc = bacc.Bacc("TRN2", target_bir_lowering=False, debug=False,
                   num_devices=NCORES, num_swdge_queues=4)
    Cc = C // 128
    ins = {
        "x": nc.dram_tensor("x", [BPC, L, C], FP32, kind="ExternalInput").ap(),
        "w5a": nc.dram_tensor("w5a", [Cc, 128, 101], FP32,
                              kind="ExternalInput").ap(),
        "w5b": nc.dram_tensor("w5b", [Cc, 128, K], FP32,
                              kind="ExternalInput").ap(),
        "wconv": nc.dram_tensor("wconv", [K * Cc, 128, F], BF16,
                                kind="ExternalInput").ap(),
        "bias": nc.dram_tensor("bias", [128, F], FP32,
                               kind="ExternalInput").ap(),
        "ident": nc.dram_tensor("ident", [128, 128], FP32,
                                kind="ExternalInput").ap(),
        "identb": nc.dram_tensor("identb", [128, 128], BF16,
                                 kind="ExternalInput").ap(),
        "iota": nc.dram_tensor("iota", [K, L], FP32,
                               kind="ExternalInput").ap(),
        "sel16": nc.dram_tensor("sel16", [8, 128, 128], FP32,
                                kind="ExternalInput").ap(),
    }
    outs = {
        "out": nc.dram_tensor("out", [BPC, L, F], FP32,
                              kind="ExternalOutput").ap(),
    }
    with tile.TileContext(nc) as tc:
        build_kernel(tc, ins, outs, Bpc=BPC, L=L, C=C, F=F, K=K,
                     cast_mode=cast_mode)
    nc.compile()
    return nc


def _prep_consts(w_off, w_conv, b_conv):
    Cc = C // 128
    w5a = np.zeros((Cc, 128, 101), np.float32)
    for t in range(4):
        for j in range(Cc):
            w5a[j, :, 32 * t:32 * t + K] = w_off[t, j * 128:(j + 1) * 128, :]
    w5b = np.zeros((Cc, 128, K), np.float32)
    for j in range(Cc):
        w5b[j] = w_off[4, j * 128:(j + 1) * 128, :]
    wconv = np.zeros((K * Cc, 128, F), ml_dtypes.bfloat16)
    for k in range(K):
        for j in range(Cc):
            wconv[k * Cc + j] = w_conv[k, j * 128:(j + 1) * 128, :].astype(
                ml_dtypes.bfloat16)
    return {
        "w5a": w5a,
        "w5b": w5b,
        "wconv": wconv,
        "bias": np.broadcast_to(
            np.asarray(b_conv, np.float32)[None, :], (128, F)).copy(),
        "iota": np.broadcast_to(
            np.arange(L, dtype=np.float32)[None, :], (K, L)).copy(),
        "ident": np.eye(128, dtype=np.float32),
        "identb": np.eye(128, dtype=ml_dtypes.bfloat16),
        "sel16": (np.arange(128)[None, :, None]
                  == 16 * np.arange(8)[:, None, None]
                  + np.arange(128)[None, None, :] % 16).astype(np.float32),
    }


def run(x, w_off, w_conv, b_conv, trace=False, trace_kwargs=None):
    x = np.ascontiguousarray(np.asarray(x, np.float32))
    assert x.shape == (B, L, C), x.shape
    if "nc" not in _CACHE:
        _CACHE["nc"] = _build_program()
    nc = _CACHE["nc"]
    consts = _prep_consts(np.asarray(w_off, np.float32),
                          np.asarray(w_conv, np.float32),
                          np.asarray(b_conv, np.float32))
    in_maps = [
        {"x": np.ascontiguousarray(x[i * BPC:(i + 1) * BPC]), **consts}
        for i in range(NCORES)
    ]
    res = run_bass_kernel_spmd(nc, in_maps, list(range(NCORES)),
                               trace=trace, **(trace_kwargs or {}))
    _CACHE["last"] = res
    out = np.concatenate([res.results[i]["out"] for i in range(NCORES)], axis=0)
    return np.ascontiguousarray(out.astype(np.float32))


def kernel(x, w_off, w_conv, b_conv):
    return run(x, w_off, w_conv, b_conv)


# revision 29
# speedup vs baseline: 1.5337x; 1.1291x over previous
"""Deformable Conv1D on 8 Trainium2 NeuronCores (Bass/Tile), batch data-parallel.

kernel(**inputs) takes the FULL inputs (x [16,4096,256] f32, w_off [5,256,5],
w_conv [5,256,512], b_conv [512]) and returns the FULL output [16,4096,512] f32.
Batch is sharded 2-per-core across 8 cores; no cross-core communication.

v2: the gather no longer uses transpose mode (which shredded each 512B row
into ~8 small hardware packets).  Rows are gathered contiguously into
[l, c] tiles and transposed to [c, l] on the PE (bf16 transposes), with the
PSUM->SBUF copies round-robined across the Scalar/Vector/Pool engines and the
main-conv matmuls software-pipelined one tile behind the transposes.

Per-core program:
  phase 1 (both batches up front): x -> SBUF; SWDGE cast-DMA writes bf16 x
    to DRAM (gather source); PE-transpose x -> xT fp32; offsets conv EXACTLY
    in fp32 (packed 25-wide stationary); DVE adds iota, clips, casts
    (truncating via the rtne -0.5 trick) to int16 indices; idx -> DRAM ->
    wrapped 16-partition x8-replica reload.
  phase 2 (per batch): for each 1024-l group, 5 no-transpose gathers land
    [128 l, 8, 256 c] bf16 tiles; per 128-l tile, 10 PE transposes make the
    [c, l] lhsT tiles, then 10-term PSUM accumulation against wconv bf16;
    DVE adds bias; DMA out.
"""

import sys

if '/opt/trn_rl_repo' not in sys.path:
    sys.path.insert(0, '/opt/trn_rl_repo')

from contextlib import ExitStack

import ml_dtypes
import numpy as np

import concourse.bass as bass
import concourse.tile as tile
from concourse import bacc, mybir
from concourse.bass_utils import run_bass_kernel_spmd

FP32 = mybir.dt.float32
BF16 = mybir.dt.bfloat16
I16 = mybir.dt.int16

B, L, C = 16, 4096, 256
F, K = 512, 5
NCORES = 8
BPC = B // NCORES  # batches per core


def build_kernel(tc, ins, outs, *, Bpc, L, C, F, K, cast_mode="rtne"):
    nc = tc.nc
    Cc = C // 128            # channel chunks (2)
    LT = L // 128            # l-tiles per batch (32)
    PAD = 4                  # zero margin around xT columns (taps reach +-2)
    WIN = 512                # offsets window width (one psum bank)
    OWN = WIN - (K - 1) - 2  # output cols owned per window (506)
    nwin = (L + OWN - 1) // OWN
    XTW = (nwin - 1) * OWN + WIN + PAD
    GRP = 1024               # gather group (l's per gather call)
    NG = L // GRP            # groups per batch (4)
    TPG = GRP // 128         # l-tiles per group (8)

    ctx = ExitStack()
    with ctx:
        const_p = ctx.enter_context(tc.tile_pool(name="const", bufs=1))
        x_p = ctx.enter_context(tc.tile_pool(name="x", bufs=1))
        xbfs_p = ctx.enter_context(tc.tile_pool(name="xbfs", bufs=2))
        xt_p = ctx.enter_context(tc.tile_pool(name="xt", bufs=1))
        xg_p = ctx.enter_context(tc.tile_pool(name="xg", bufs=2))
        xgt_p = ctx.enter_context(tc.tile_pool(name="xgt", bufs=6))
        idx_p = ctx.enter_context(tc.tile_pool(name="idx", bufs=2))
        posm_p = ctx.enter_context(tc.tile_pool(name="posm", bufs=1))
        acc_p = ctx.enter_context(tc.tile_pool(name="acc", bufs=2))
        out_p = ctx.enter_context(tc.tile_pool(name="out", bufs=3))
        ps_t = ctx.enter_context(tc.tile_pool(name="ps_t", bufs=2, space="PSUM"))
        ps_g = ctx.enter_context(tc.tile_pool(name="ps_g", bufs=2, space="PSUM"))
        ps_o = ctx.enter_context(tc.tile_pool(name="ps_o", bufs=2, space="PSUM"))
        ps_m = ctx.enter_context(tc.tile_pool(name="ps_m", bufs=2, space="PSUM"))
        dram_p = ctx.enter_context(tc.tile_pool(name="dram", bufs=1, space="DRAM"))

        # resident constants: w5a packs taps 0..3 at 32-aligned col groups
        # (w5a[c, 32t+k] = w_off[t, c, k]); w5b is tap 4.
        w5a_sb = const_p.tile([128, Cc, 101], FP32, tag="w5a")
        nc.sync.dma_start(w5a_sb[:], ins["w5a"][:].rearrange("j p k -> p j k"))
        w5b_sb = const_p.tile([128, Cc, K], FP32, tag="w5b")
        nc.sync.dma_start(w5b_sb[:], ins["w5b"][:].rearrange("j p k -> p j k"))
        wconv_sb = const_p.tile([128, K * Cc, F], BF16, tag="wconv")
        nc.sync.dma_start(wconv_sb[:], ins["wconv"][:].rearrange("q p f -> p q f"))
        bias_sb = const_p.tile([128, F], FP32, tag="bias")
        nc.sync.dma_start(bias_sb[:], ins["bias"][:])
        iota_sb = const_p.tile([K, L], FP32, tag="iota")
        nc.sync.dma_start(iota_sb[:], ins["iota"][:])
        ident_sb = const_p.tile([128, 128], FP32, tag="ident")
        nc.sync.dma_start(ident_sb[:], ins["ident"][:])
        identb_sb = const_p.tile([128, 128], BF16, tag="identb")
        nc.sync.dma_start(identb_sb[:], ins["identb"][:])
        sel16_sb = const_p.tile([128, 8, 128], FP32, tag="sel16")
        nc.sync.dma_start(sel16_sb[:], ins["sel16"][:].rearrange("g p m -> p g m"))

        xbf = dram_p.tile([Bpc, L, C], BF16, tag="xbf")

        idx_ts = {}

        # ---------------- phase 1: both batches ----------------
        for b in range(Bpc):
            # ---- load x[b] in halves: [128 (l%128), LT/2, C] ----
            xt = [xt_p.tile([128, XTW], FP32, tag=f"xt{j}", name=f"xt{j}_{b}")
                  for j in range(Cc)]
            for j in range(Cc):
                nc.vector.memset(xt[j][:, 0:PAD], 0.0)
                nc.vector.memset(xt[j][:, PAD + L:XTW], 0.0)
            LH = LT // 2
            for xh in range(2):
                x_sb = x_p.tile([128, LH, C], FP32, tag="x",
                                name=f"x_{b}_{xh}")
                nc.sync.dma_start(
                    x_sb[:],
                    ins["x"][b, xh * (L // 2):(xh + 1) * (L // 2)]
                    .rearrange("(t p) c -> p t c", p=128))
                # bf16 copy in DRAM (gather source): Pool-engine cast to a
                # small staging tile + HWDGE store, keeping SWDGE queues free
                # for the gathers.
                for cq in range(4):
                    xbf_sb = xbfs_p.tile([128, 4, C], BF16, tag="xbf_sb",
                                         name=f"xbf_sb_{b}_{xh}_{cq}")
                    nc.gpsimd.tensor_copy(
                        xbf_sb[:], x_sb[:, 4 * cq:4 * (cq + 1), :])
                    nc.sync.dma_start(
                        out=xbf[b, xh * (L // 2):(xh + 1) * (L // 2)]
                        .rearrange("(t p) c -> p t c", p=128)[:,
                                                             4 * cq:4 * (cq + 1)],
                        in_=xbf_sb[:])
                # PE-transpose x -> xT[j][c, PAD + l] (fp32)
                for j in range(Cc):
                    for g4 in range(LH // 4):
                        g4g = xh * (LH // 4) + g4
                        pst4 = ps_t.tile([128, 4, 128], FP32, tag="pst")
                        for i4 in range(4):
                            nc.tensor.transpose(
                                pst4[:, i4, :],
                                x_sb[:, 4 * g4 + i4, j * 128:(j + 1) * 128],
                                ident_sb[:])
                        nc.scalar.copy(
                            xt[j][:, PAD + 512 * g4g:PAD + 512 * (g4g + 1)]
                            .rearrange("p (a c) -> p a c", a=4),
                            pst4[:])

            # ---- offsets windows -> idx [K, L] int16 (fp32-exact) ----
            # taps 0-3 land at psum partition groups {0,32,64,96}+[0,5);
            # tap 4 is accumulated onto tap 0's frame by a second matmul
            # whose moving slice is shifted +4 columns.
            posm_sb = posm_p.tile([K, L], FP32, tag="posm", name=f"posm_{b}")
            for s in range(nwin):
                o = s * OWN
                W = min(OWN, L - o)
                ps = ps_o.tile([128, WIN], FP32, tag="ps25")
                for j in range(Cc):
                    nc.tensor.matmul(
                        ps[0:101, :], w5a_sb[:, j, :], xt[j][:, o:o + WIN],
                        start=(j == 0), stop=(j == Cc - 1))
                for j in range(Cc):
                    nc.tensor.matmul(
                        ps[0:K, 0:WIN - 4], w5b_sb[:, j, :],
                        xt[j][:, o + 4:o + WIN], start=False,
                        stop=(j == Cc - 1), skip_group_check=True)
                acc = acc_p.tile([K, OWN], FP32, tag="acc")
                nc.vector.tensor_tensor(
                    out=acc[:, :W], in0=iota_sb[:, o:o + W],
                    in1=ps[0:K, 2:2 + W], op=mybir.AluOpType.add)
                for t, sh in ((32, 3), (64, 4), (96, 5)):
                    nc.vector.tensor_tensor(
                        out=acc[:, :W], in0=acc[:, :W],
                        in1=ps[t:t + K, sh:sh + W], op=mybir.AluOpType.add)
                if cast_mode == "rtne":
                    # HW float->int cast rounds to nearest even; emulate the
                    # reference's truncation via floor = rtne(clip(pos) - 0.5).
                    # Asymmetric clip bounds keep the clipped values off the
                    # rounding-half boundaries (0.25-0.5=-0.25 -> 0;
                    # (L-1)+0.25-0.5 -> L-1).  Kept in fp32 here; the int16
                    # cast happens after the wrapped reload.
                    nc.vector.tensor_scalar(
                        out=acc[:, :W], in0=acc[:, :W],
                        scalar1=0.25, scalar2=float(L - 1) + 0.25,
                        op0=mybir.AluOpType.max, op1=mybir.AluOpType.min)
                    nc.vector.tensor_scalar(
                        out=posm_sb[:, o:o + W], in0=acc[:, :W],
                        scalar1=-0.5, scalar2=None, op0=mybir.AluOpType.add)
                else:
                    # CoreSim float->int cast truncates toward zero.
                    nc.vector.tensor_scalar(
                        out=posm_sb[:, o:o + W], in0=acc[:, :W],
                        scalar1=0.0, scalar2=float(L - 1),
                        op0=mybir.AluOpType.max, op1=mybir.AluOpType.min)

            # ---- positions -> gather-index tiles, all on-chip ----
            # 1) PE-transpose posm [5, L] in 128-chunks -> U [128 (l%128), K, LT]
            u_sb = idx_p.tile([128, K, LT], FP32, tag="u", name=f"u_{b}")
            for t4 in range(LT // 4):
                psu = ps_t.tile([128, 4, 128], FP32, tag="pst",
                                name=f"psu_{b}_{t4}")
                for i4 in range(4):
                    nc.tensor.transpose(
                        psu[:, i4, 0:K],
                        posm_sb[:, (4 * t4 + i4) * 128:(4 * t4 + i4 + 1) * 128],
                        ident_sb[0:K, 0:K])
                nc.scalar.copy(
                    u_sb[:, :, 4 * t4:4 * (t4 + 1)],
                    psu[:, 0:4, 0:K].rearrange("p a k -> p k a"))
            # 2) per (g8 residue group, h half): replicate U's 16-partition
            #    slice to 128 partitions via sel16 matmul (exact 1.0*v), then
            #    cast to int16 (rtne -0.5 trick already applied in posm).
            for g8 in range(8):
                for q in range(4):
                    psr = ps_t.tile([128, 4, 128], FP32, tag="pst",
                                    name=f"psr_{b}_{g8}_{q}")
                    nc.tensor.matmul(
                        psr[:, 0, 0:K * 8],
                        sel16_sb[:, g8, :],
                        u_sb[:, :, 8 * q:8 * (q + 1)],
                        start=True, stop=True)
                    idx_j = idx_p.tile([128, K * 8], I16, tag=f"idxJ{g8}_{q}",
                                       name=f"idxJ{g8}_{q}_{b}")
                    nc.vector.tensor_copy(idx_j[:], psr[:, 0, 0:K * 8])
                    idx_ts[b, g8, q] = idx_j

        # ---------------- phase 2: gather + transpose + main conv ----------------
        # Gather (g8, h): num_idxs = 1280 covers taps k=0..4 x t' in
        # [16h, 16h+16); descriptor i = 256k + 16t' + pp targets row
        # l = 128(16h + t') + 16 g8 + pp, landing at out[16(t'%8) + pp,
        # slot 2k + t'//8, :].  Tile (g8, j): rows l = 1024j + 128uu +
        # 16 g8 + pp at partition 16uu + pp, from slot 2k + j%2 of half j//2.
        NIDX = K * 16 * 8           # 640 indices per gather call
        NSLOT = NIDX // 128         # 5 output slots (= taps)

        def emit_gathers(b, h):
            xg = []
            for g8 in range(8):
                xgk = xg_p.tile([128, NSLOT, C], BF16, tag=f"xg{g8}",
                                name=f"xg{g8}_{b}_{h}")
                nc.gpsimd.dma_gather(
                    out_ap=xgk[:], in_ap=xbf[b],
                    idxs_ap=idx_ts[b, g8, h][:],
                    num_idxs=NIDX, num_idxs_reg=NIDX,
                    elem_size=C, transpose=False, single_packet=True,
                    queue_num=0)
                xg.append(xgk)
            return xg

        def emit_transposes(b, q, g8):
            xg = cur_xg[q % 2][g8]
            ts = []
            for half in range(2):
                pg = ps_g.tile([128, K, 128], BF16, tag="psg")
                for m5 in range(K):
                    m = half * K + m5
                    k, j2 = m // Cc, m % Cc
                    nc.tensor.transpose(
                        pg[:, m5, :],
                        xg[:, k, j2 * 128:(j2 + 1) * 128],
                        identb_sb[:])
                xgt = xgt_p.tile([128, K, 128], BF16, tag="xgt",
                                 name=f"xgt{half}_{b}_{g8}_{q}")
                if half == 0:
                    nc.scalar.copy(xgt[:], pg[:])
                else:
                    nc.vector.tensor_copy(xgt[:], pg[:])
                ts.append(xgt)
            return ts

        out_v = outs["out"]

        def emit_matmuls(b, g8, j, ts):
            pso = ps_m.tile([128, F], FP32, tag="pso")
            for m in range(K * Cc):
                nc.tensor.matmul(
                    pso[:], ts[m // K][:, m % K, :], wconv_sb[:, m, :],
                    start=(m == 0), stop=(m == K * Cc - 1))
            o_sb = out_p.tile([128, F], FP32, tag="osb")
            nc.vector.tensor_tensor(
                out=o_sb[:], in0=pso[:], in1=bias_sb[:],
                op=mybir.AluOpType.add)
            dst = out_v[b].rearrange(
                "(j uu g pp) f -> j g uu pp f", uu=8, g=8, pp=16)[j, g8]
            nc.sync.dma_start(dst, o_sb[:])

        for b in range(Bpc):
            cur_xg = [None, None]
            cur_xg[0] = emit_gathers(b, 0)
            pending = None  # (g8, j, transposed tiles)
            NQ = LT // 8    # quarters per batch (4)
            for q in range(NQ):
                if q + 1 < NQ:
                    cur_xg[(q + 1) % 2] = emit_gathers(b, q + 1)
                for g8 in range(8):
                    ts = emit_transposes(b, q, g8)
                    if pending is not None:
                        emit_matmuls(b, pending[0], pending[1], pending[2])
                    pending = (g8, q, ts)
            emit_matmuls(b, pending[0], pending[1], pending[2])


_CACHE = {}


def _build_program(cast_mode="rtne"):
    nc = bacc.Bacc("TRN2", target_bir_lowering=False, debug=False,
                   num_devices=NCORES, num_swdge_queues=4)
    Cc = C // 128
    ins = {
        "x": nc.dram_tensor("x", [BPC, L, C], FP32, kind="ExternalInput").ap(),
        "w5a": nc.dram_tensor("w5a", [Cc, 128, 101], FP32,
                              kind="ExternalInput").ap(),
        "w5b": nc.dram_tensor("w5b", [Cc, 128, K], FP32,
                              kind="ExternalInput").ap(),
        "wconv": nc.dram_tensor("wconv", [K * Cc, 128, F], BF16,
                                kind="ExternalInput").ap(),
        "bias": nc.dram_tensor("bias", [128, F], FP32,
                               kind="ExternalInput").ap(),
        "ident": nc.dram_tensor("ident", [128, 128], FP32,
                                kind="ExternalInput").ap(),
        "identb": nc.dram_tensor("identb", [128, 128], BF16,
                                 kind="ExternalInput").ap(),
        "iota": nc.dram_tensor("iota", [K, L], FP32,
                               kind="ExternalInput").ap(),
        "sel16": nc.dram_tensor("sel16", [8, 128, 128], FP32,
                                kind="ExternalInput").ap(),
    }
    outs = {
        "out": nc.dram_tensor("out", [BPC, L, F], FP32,
                              kind="ExternalOutput").ap(),
    }
    with tile.TileContext(nc) as tc:
        build_kernel(tc, ins, outs, Bpc=BPC, L=L, C=C, F=F, K=K,
                     cast_mode=cast_mode)
    _spread_gather_queues(nc)
    nc.compile()
    return nc


def _spread_gather_queues(nc):
    """Distribute SWDGE gathers across the 4 queues.

    The tile framework round-robins Pool-DMA completion sems over 8 DMASW
    lanes in final block order; a sem lane must only ever be updated from one
    queue, so queue = (block_index % 8) % 4 keeps each lane on a fixed queue
    while getting 4-way DMA parallelism."""
    i = 0
    for fn in nc.m.functions:
        for bb in fn.blocks:
            for inst in bb.instructions:
                if type(inst).__name__ == "InstDMAGatherAnt":
                    inst.queue_num = (i % 8) % 4
                    i += 1


def _prep_consts(w_off, w_conv, b_conv):
    Cc = C // 128
    w5a = np.zeros((Cc, 128, 101), np.float32)
    for t in range(4):
        for j in range(Cc):
            w5a[j, :, 32 * t:32 * t + K] = w_off[t, j * 128:(j + 1) * 128, :]
    w5b = np.zeros((Cc, 128, K), np.float32)
    for j in range(Cc):
        w5b[j] = w_off[4, j * 128:(j + 1) * 128, :]
    wconv = np.zeros((K * Cc, 128, F), ml_dtypes.bfloat16)
    for k in range(K):
        for j in range(Cc):
            wconv[k * Cc + j] = w_conv[k, j * 128:(j + 1) * 128, :].astype(
                ml_dtypes.bfloat16)
    return {
        "w5a": w5a,
        "w5b": w5b,
        "wconv": wconv,
        "bias": np.broadcast_to(
            np.asarray(b_conv, np.float32)[None, :], (128, F)).copy(),
        "iota": np.broadcast_to(
            np.arange(L, dtype=np.float32)[None, :], (K, L)).copy(),
        "ident": np.eye(128, dtype=np.float32),
        "identb": np.eye(128, dtype=ml_dtypes.bfloat16),
        "sel16": (np.arange(128)[None, :, None]
                  == 16 * np.arange(8)[:, None, None]
                  + np.arange(128)[None, None, :] % 16).astype(np.float32),
    }


def run(x, w_off, w_conv, b_conv, trace=False, trace_kwargs=None):
    x = np.ascontiguousarray(np.asarray(x, np.float32))
    assert x.shape == (B, L, C), x.shape
    if "nc" not in _CACHE:
        _CACHE["nc"] = _build_program()
    nc = _CACHE["nc"]
    consts = _prep_consts(np.asarray(w_off, np.float32),
                          np.asarray(w_conv, np.float32),
                          np.asarray(b_conv, np.float32))
    in_maps = [
        {"x": np.ascontiguousarray(x[i * BPC:(i + 1) * BPC]), **consts}
        for i in range(NCORES)
    ]
    res = run_bass_kernel_spmd(nc, in_maps, list(range(NCORES)),
                               trace=trace, **(trace_kwargs or {}))
    _CACHE["last"] = res
    out = np.concatenate([res.results[i]["out"] for i in range(NCORES)], axis=0)
    return np.ascontiguousarray(out.astype(np.float32))


def kernel(x, w_off, w_conv, b_conv):
    return run(x, w_off, w_conv, b_conv)
